# revision 9
# baseline (speedup 1.0000x reference)
"""Trainium2 Bass kernel for nn_Attention_66314295050336.

Sparse (threshold-pruned) multi-head attention:
    qkv  = x @ w_qkv.T + b_qkv          [B,N,3C]
    q,k,v heads (H=6, D=64), attn = softmax(mask(q@k.T * D**-0.5))
    mask: scores < 0.0 -> -10000 before softmax (=> weight 0 in fp32)
    out  = (attn @ v) @ w_proj.T + b_proj

Sharding: pure data-parallel over batch B=8 across the 8 NeuronCores
(one batch per core, no collectives).  Per core everything stays
SBUF-resident; all matmuls are fp16 with fp32 PSUM accumulation:

  xT[c,n] --(fp16 MM)--> qkT [768,1024] (heads paired on partitions)
                     +-> v   [1024,384] (fp16)
  scoresT[k,q] per head pair via row-packed K=64 matmuls
  ACT exp(scale*s) psum->SBUF fp16; threshold mask on DVE batched over
  4 kt blocks: b = (e >= 1) [tensor_scalar 4x]; e *= b [TT 2x_1p]
  (e >= 1  <=>  score >= 0, measure-zero edge at exactly 0)
  attn@v: col-packed M=64 pairs, lhsT=v; a parallel ones-matmul gives
  Z (softmax denominator) replicated across 64 partitions, so
  normalization is reciprocal_approx_fast + one tensor_tensor multiply.
  AV/Z matmuls drain from a pending queue one tick after their mask is
  queued so the PE FIFO never head-of-line blocks on the DVE chain.
  proj: fp16 matmuls from SBUF, copied PSUM->SBUF, DMAed to HBM.

Engine budget (per core): Scalar = 48 exps (~54us, pace-setter with
DVE), DVE = masks (~48us) + production casts + norm, PE ~62us busy.
GPSIMD offload was measured and REGRESSES (POOL SBUF port is shared
with DVE: concurrent gpsimd TT halves DVE throughput).

Biases are zeros per the problem spec (fill: zeros); asserted below.
"""

import os
import sys

import numpy as np

for _p in ("/opt/trn_rl_repo", "/root/.axon_site/_ro/trn_rl_repo"):
    if os.path.isdir(_p) and _p not in sys.path:
        sys.path.insert(0, _p)

N = 1024
C = 384
H = 6
D = 64
SCALE = float(D) ** -0.5  # 0.125
NCORES = 8

_CACHE = {}


def _build():
    import concourse.bass as bass
    import concourse.mybir as mybir
    import concourse.tile as tile
    from concourse import bacc
    from contextlib import ExitStack

    F32 = mybir.dt.float32
    F16 = mybir.dt.float16
    MULT = mybir.AluOpType.mult
    IS_GE = mybir.AluOpType.is_ge
    EXP = mybir.ActivationFunctionType.Exp

    nc = bacc.Bacc(
        "TRN2", target_bir_lowering=False, debug=False, enable_asserts=False
    )

    xT_d = nc.dram_tensor("xT", [C, N], F16, kind="ExternalInput")
    wqkvT_d = nc.dram_tensor("wqkvT", [C, 3 * C], F16, kind="ExternalInput")
    wprojT_d = nc.dram_tensor("wprojT", [C, C], F16, kind="ExternalInput")
    out_d = nc.dram_tensor("out", [N, C], F32, kind="ExternalOutput")

    with tile.TileContext(nc) as tc, ExitStack() as ctx:
        const = ctx.enter_context(tc.tile_pool(name="const", bufs=1))
        epool = ctx.enter_context(tc.tile_pool(name="e", bufs=2))
        bpool = ctx.enter_context(tc.tile_pool(name="bn", bufs=2))
        psS = ctx.enter_context(
            tc.tile_pool(name="psS", bufs=2, space=bass.MemorySpace.PSUM)
        )
        psO = ctx.enter_context(
            tc.tile_pool(name="psO", bufs=1, space=bass.MemorySpace.PSUM)
        )

        xT = const.tile([128, 3 * N], F16)  # c-tile ct -> cols [ct*N:(ct+1)*N]
        wqkv = const.tile([128, 3 * 3 * C], F16)  # ct -> cols [ct*1152 ...]
        wproj = const.tile([128, 3 * C], F16)
        qk = const.tile([128, 6 * N], F16)  # q pairs 0..2, k pairs 3..5
        v = const.tile([128, 8 * 512], F16)  # n-tile nt -> cols [nt*512 ...+384]
        ones64 = const.tile([128, 64], F16)
        outT = const.tile([128, 3 * N], F16)  # pair p -> cols [p*N:(p+1)*N]

        # input DMAs, critical-first across the two DMA-capable queues:
        # wave 1 = xT + wqkv-q chunks (first qk pair), wave 2 = wqkv-k
        # (v production), wave 3 = wproj (needed only at the tail).
        def dma_xT(ct, eng):
            r = slice(ct * 128, (ct + 1) * 128)
            eng.dma_start(xT[:, ct * N : (ct + 1) * N], xT_d[r, :])

        def dma_wq(ct, eng):
            r = slice(ct * 128, (ct + 1) * 128)
            eng.dma_start(
                wqkv[:, ct * 1152 : ct * 1152 + 768], wqkvT_d[r, 0:768]
            )

        def dma_wk(ct, eng):
            r = slice(ct * 128, (ct + 1) * 128)
            eng.dma_start(
                wqkv[:, ct * 1152 + 768 : (ct + 1) * 1152], wqkvT_d[r, 768:]
            )

        def dma_wp(ct, eng):
            r = slice(ct * 128, (ct + 1) * 128)
            eng.dma_start(wproj[:, ct * C : (ct + 1) * C], wprojT_d[r, :])

        dma_xT(0, nc.sync)
        dma_wq(0, nc.scalar)
        dma_wq(1, nc.sync)
        dma_xT(1, nc.scalar)
        dma_xT(2, nc.sync)
        dma_wq(2, nc.scalar)
        dma_wk(0, nc.sync)
        dma_wk(1, nc.scalar)
        dma_wk(2, nc.sync)
        dma_wp(0, nc.scalar)
        dma_wp(1, nc.sync)
        dma_wp(2, nc.scalar)
        nc.gpsimd.memset(ones64[:], 1.0)

        # ---------------- production: paired groups, one batched copy -----
        def fill_tile():
            # single [128,1024] PSUM slot (2 banks) for background groups
            return psS.tile([128, 1024], F32, tag="f", bufs=1, name="fill")

        def emit_qk_pair(oc, init_eng=None):
            # both nh halves of o-chunk oc into one psum tile, one copy
            ps = (
                psS.tile([128, 1024], F32, tag="s", name=f"init_{oc}")
                if init_eng is not None
                else fill_tile()
            )
            for nh in range(2):
                for ct in range(3):
                    nc.tensor.matmul(
                        ps[:, nh * 512 : (nh + 1) * 512],
                        wqkv[
                            :, ct * 1152 + oc * 128 : ct * 1152 + (oc + 1) * 128
                        ],
                        xT[:, ct * N + nh * 512 : ct * N + nh * 512 + 512],
                        start=(ct == 0),
                        stop=(ct == 2),
                    )
            dst = qk[:, oc * N : oc * N + 1024]
            if init_eng is nc.scalar:
                nc.scalar.copy(dst, ps[:, 0:1024])
            else:
                nc.vector.tensor_copy(dst, ps[:, 0:1024])

        def emit_v_pair(nt):
            # v tiles nt, nt+1 into one psum tile, one copy
            ps = fill_tile()
            for j in range(2):
                for ct in range(3):
                    nc.tensor.matmul(
                        ps[:, j * 512 : j * 512 + C],
                        xT[:, ct * N + (nt + j) * 128 : ct * N + (nt + j + 1) * 128],
                        wqkv[:, ct * 1152 + 768 : ct * 1152 + 1152],
                        start=(ct == 0),
                        stop=(ct == 2),
                    )
            # one batched copy; cols [C:512) of each half are never read
            nc.vector.tensor_copy(
                v[:, nt * 512 : (nt + 2) * 512], ps[:, 0:1024]
            )

        emit_qk_pair(0, init_eng=nc.scalar)  # q pair 0
        emit_qk_pair(3, init_eng=nc.vector)  # k pair 0

        background = [
            lambda: emit_v_pair(0), lambda: emit_v_pair(2),
            lambda: emit_v_pair(4), lambda: emit_v_pair(6),
            lambda: emit_qk_pair(1), lambda: emit_qk_pair(4),
            lambda: emit_qk_pair(2), lambda: emit_qk_pair(5),
        ]

        # ---------------- attention, head pairs, qc-outer -----------------
        # e_pair layout: block bi=(qc*8+kt) -> cols [bi*1024 : bi*1024+1024],
        # block = [h_even 512 | h_odd 512] for that (kt, qc).
        pending = []  # deferred AV/Z work items, drained <=2 per kt tick

        def drain(n):
            for _ in range(min(n, len(pending))):
                it = pending.pop(0)
                (qc, kb, e_pair, O_ps, Z_ps, h_ev, h_od, norm_fn) = it
                bj = qc * 8 + kb
                rev = e_pair[:, bj * 1024 : bj * 1024 + 512]
                rod = e_pair[:, bj * 1024 + 512 : bj * 1024 + 1024]
                vev = v[:, kb * 512 + h_ev * 64 : kb * 512 + h_ev * 64 + 64]
                vod = v[:, kb * 512 + h_od * 64 : kb * 512 + h_od * 64 + 64]
                st, sp = (kb == 0), (kb == 7)
                # outT_h[d,q] accumulation, two heads col-packed
                nc.tensor.matmul(
                    O_ps[0:64, :], vev, rev, start=st, stop=sp,
                    tile_position=(0, 0), skip_group_check=True,
                )
                nc.tensor.matmul(
                    O_ps[64:128, :], vod, rod, start=st, stop=sp,
                    tile_position=(0, 64), skip_group_check=True,
                )
                # Z_h[q] (replicated x64): ones-matmul, same rhs
                nc.tensor.matmul(
                    Z_ps[0:64, :], ones64[:, 0:64], rev, start=st,
                    stop=sp, tile_position=(0, 0), skip_group_check=True,
                )
                nc.tensor.matmul(
                    Z_ps[64:128, :], ones64[:, 0:64], rod, start=st,
                    stop=sp, tile_position=(0, 64), skip_group_check=True,
                )
                if sp:
                    norm_fn()

        for p in range(3):
            h_ev, h_od = 2 * p, 2 * p + 1
            e_pair = epool.tile([128, 16 * N], F16, tag="e", name=f"e_{p}")
            qT0 = p * N
            kT0 = (3 + p) * N
            for qc in range(2):
                O_ps = psO.tile([128, 512], F32, tag="O", name=f"O_{p}_{qc}")
                Z_ps = psO.tile([128, 512], F32, tag="Zb", name=f"Z_{p}_{qc}")
                last_qc = p == 2 and qc == 1

                def make_norm(p=p, qc=qc, O_ps=O_ps, Z_ps=Z_ps, last=last_qc):
                    def norm_fn():
                        B = bpool.tile(
                            [128, 512], F32, tag="B", name=f"B_{p}_{qc}"
                        )
                        dst = outT[:, p * N + qc * 512 : p * N + qc * 512 + 512]
                        if last:
                            nc.vector.reciprocal_approx_fast(B[:], Z_ps[:])
                            nc.vector.tensor_mul(dst, O_ps[:], B[:])
                        else:
                            # deprioritized: recip/norm wait on the trailing
                            # Z flush; don't head-of-line block the masks
                            with tc.high_priority(offset=-10):
                                nc.vector.reciprocal_approx_fast(B[:], Z_ps[:])
                                nc.vector.tensor_mul(dst, O_ps[:], B[:])

                    return norm_fn

                norm_fn = make_norm()
                grp = 2 if last_qc else 4
                for kt in range(8):
                    if kt % 2 == 0 and background:
                        background.pop(0)()
                    bi = qc * 8 + kt
                    s = psS.tile([128, 1024], F32, tag="s", name=f"s_{p}_{bi}")
                    # scoresT[k,q] = sum_d kT[d,k]*qT[d,q]; heads row-packed
                    nc.tensor.matmul(
                        s[:, 0:512],
                        qk[0:64, kT0 + kt * 128 : kT0 + (kt + 1) * 128],
                        qk[0:64, qT0 + qc * 512 : qT0 + (qc + 1) * 512],
                        start=True,
                        stop=True,
                    )
                    nc.tensor.matmul(
                        s[:, 512:1024],
                        qk[64:128, kT0 + kt * 128 : kT0 + (kt + 1) * 128],
                        qk[64:128, qT0 + qc * 512 : qT0 + (qc + 1) * 512],
                        start=True,
                        stop=True,
                    )
                    eb = e_pair[:, bi * 1024 : (bi + 1) * 1024]
                    nc.scalar.activation(eb, s[:], EXP, scale=SCALE)
                    new_kbs = []
                    if kt % grp == grp - 1:
                        e2 = e_pair[
                            :, (bi - grp + 1) * 1024 : (bi + 1) * 1024
                        ]
                        b = bpool.tile(
                            [128, grp * 1024], F16, tag="b", name=f"b_{p}_{bi}"
                        )
                        nc.vector.tensor_scalar(b[:], e2, 1.0, None, IS_GE)
                        nc.vector.tensor_mul(e2, e2, b[:])
                        new_kbs = list(range(kt - grp + 1, kt + 1))
                    # drain BEFORE queueing this tick's kbs: AV matmuls only
                    # reference masks queued at least one tick earlier
                    drain(2)
                    for kb in new_kbs:
                        pending.append(
                            (qc, kb, e_pair, O_ps, Z_ps, h_ev, h_od, norm_fn)
                        )

        while pending:
            drain(2)

        # ---------------- proj + store ------------------------------------
        # s slots are free now; rotate proj psum through s,s,f (3-deep)
        for qt in range(8):
            if qt % 3 == 2:
                ps = fill_tile()
            else:
                ps = psS.tile([128, 1024], F32, tag="s", name=f"proj_{qt}")
            for p3 in range(3):
                nc.tensor.matmul(
                    ps[:, 0:C],
                    outT[:, p3 * N + qt * 128 : p3 * N + (qt + 1) * 128],
                    wproj[:, p3 * C : (p3 + 1) * C],
                    start=(p3 == 0),
                    stop=(p3 == 2),
                )
            fin = bpool.tile([128, C], F32, tag="fin", name=f"fin_{qt}")
            if qt % 2 == 0:
                nc.scalar.copy(fin[:], ps[:, 0:C])
            else:
                nc.vector.tensor_copy(fin[:], ps[:, 0:C])
            nc.sync.dma_start(out_d[qt * 128 : (qt + 1) * 128, :], fin[:])

    nc.compile()
    return nc


def get_nc():
    if "nc" not in _CACHE:
        _CACHE["nc"] = _build()
    return _CACHE["nc"]


def make_in_maps(x, w_qkv, w_proj):
    wqkvT = np.ascontiguousarray(w_qkv.T).astype(np.float16)
    wprojT = np.ascontiguousarray(w_proj.T).astype(np.float16)
    return [
        {
            "xT": np.ascontiguousarray(x[b].T).astype(np.float16),
            "wqkvT": wqkvT,
            "wprojT": wprojT,
        }
        for b in range(x.shape[0])
    ]


def kernel(x, w_qkv, b_qkv, w_proj, b_proj):
    from concourse.bass_utils import run_bass_kernel_spmd

    x = np.asarray(x)
    assert x.shape == (NCORES, N, C), x.shape
    assert not np.asarray(b_qkv).any() and not np.asarray(b_proj).any(), (
        "kernel specialized for zero biases (problem spec fill=zeros)"
    )

    nc = get_nc()
    res = run_bass_kernel_spmd(nc, make_in_maps(x, w_qkv, w_proj), list(range(NCORES)))
    out = np.stack([res.results[i]["out"] for i in range(NCORES)], axis=0)
    return out.astype(np.float32)


if __name__ == "__main__":
    nc = get_nc()
    print("built + compiled OK:", nc)
